# revision 13
# baseline (speedup 1.0000x reference)
"""BiGCN layer kernel for 8 Trainium2 NeuronCores.

Strategy (1D column-parallel SpMM, fp8e3 adjacency stream, ReduceScatter
epilogue):
  - Each core c owns the contraction slice n in [c*512, (c+1)*512) of all six
    adjacency matrices (3 bw + 3 fw), pre-transposed AND pre-swizzled on host
    to the exact SBUF tile layout so every DMA is 128 x contiguous-4KB lines
    (no rearrange descriptor storms).
  - Adjacency is stored as e3m4 fp8 of 16*(a - 0.5): the 0.5 shift centers the
    uniform[0,1) values (halving quantization noise) and the x16 scale moves
    the payload into e3m4's normal range (subnormal-flush safe). The dropped
    mean term 0.5*sum_n sup[n,k] telescopes across cores and relations into a
    single per-h constant, which the host folds into the pre-relu bias.
  - sup'[r] = inps @ (W[r]/16) is computed locally per core for its n-slice
    (fp16), so 16*(a-.5) @ sup' == (a-.5) @ sup with no rescale.
  - feats^T partials (all m, summed over a direction's 3 relations in PSUM)
    stage to DRAM in fp16 and ReduceScatter across the 8 cores; RS(bw)
    overlaps the fw stream. Core c receives its own m-block.
  - bias+relu fuse into one scalar-engine activation (bias is per-partition
    because feats is produced transposed [h, m]); the final linear runs in
    fp32r, accumulated per h-row so its bw half overlaps RS(fw); the residual
    reuses the fp16 inps^T tile already on SBUF (3e-5 output error).
"""

import numpy as np
import ml_dtypes

N, H, R = 4096, 512, 3
K = H // 2            # 256
NC = 8                # cores
NB = N // NC          # 512 rows (m / n_loc) per core
MC = 512              # m-chunk width streamed per PSUM accumulation group
HT = H // 128         # 4 h-tiles
NT = NB // 128        # 4 n_loc tiles
JT = H // 128         # 4 output j tiles
NMC = N // MC         # 4 m chunks
ASCALE = 16.0         # adjacency fp8 scale: q = e3m4(16*(a-0.5))

# adjacency on-chip dtype: "f8e3" (half DMA, mixed-dtype matmul) or "f16"
ADJ_MODE = "f8e3"

_BUILT = {}


def _build_nc():
    """Build (and cache) the Bass program. Identical program on all 8 cores."""
    if "nc" in _BUILT:
        return _BUILT["nc"]

    import concourse.bass as bass
    import concourse.mybir as mybir
    from concourse import bacc, tile

    f32 = mybir.dt.float32
    f32r = mybir.dt.float32r
    f16 = mybir.dt.float16
    adt = mybir.dt.float8e3 if ADJ_MODE == "f8e3" else f16
    nc = bacc.Bacc(None, num_devices=NC)

    inpsT = nc.dram_tensor("inpsT", [128, HT, NB], f16, kind="ExternalInput")
    wst = nc.dram_tensor("wst", [128, 2 * R, HT, K], f16, kind="ExternalInput")
    adjq = nc.dram_tensor("adjq", [2 * R, NMC, 128, NT, MC], adt, kind="ExternalInput")
    biases = nc.dram_tensor("biases", [128, JT, 2], f32, kind="ExternalInput")
    w1 = nc.dram_tensor("w1", [128, HT, H], f32r, kind="ExternalInput")
    ident = nc.dram_tensor("ident", [128, 128], f16, kind="ExternalInput")
    outT = nc.dram_tensor("outT", [128, JT, NB], f16, kind="ExternalOutput")

    Relu = mybir.ActivationFunctionType.Relu
    Identity = mybir.ActivationFunctionType.Identity

    with tile.TileContext(nc) as tc:
        with (
            tc.tile_pool(name="const", bufs=1) as const,
            tc.tile_pool(name="adjp", bufs=9) as adjp,
            tc.tile_pool(name="evacp", bufs=3) as evacp,
            tc.tile_pool(name="psum", bufs=4, space=bass.MemorySpace.PSUM) as psump,
            tc.tile_pool(name="dram", bufs=1, space="DRAM") as dramp,
        ):
            # ---------------- constants into SBUF (all pre-swizzled) --------
            # inpsT/wst split per-ht so the first sup matmul's inputs land
            # after ~300KB instead of the full 1.25MB. The fw weight half and
            # epilogue constants ride the scalar HWDGE ring, which the
            # adjacency stream (sync ring) never blocks.
            inpsT_sb = const.tile([128, HT, NB], f16)       # [p_h, ht, n_loc]
            wst_sb = const.tile([128, 2 * R, HT, K], f16)   # [p_h, r, ht, k]
            for ht in range(HT):
                nc.sync.dma_start(inpsT_sb[:, ht], inpsT[:, ht])
                nc.sync.dma_start(wst_sb[:, 0:R, ht], wst[:, 0:R, ht])
            nc.scalar.dma_start(wst_sb[:, R : 2 * R], wst[:, R : 2 * R])
            biases_sb = const.tile([128, JT, 2], f32)
            nc.scalar.dma_start(biases_sb[:], biases[:, :, :])
            w1_sb = const.tile([128, HT, H], f32r)          # [p_h, ht, j]
            nc.scalar.dma_start(w1_sb[:], w1[:, :, :])
            ident_sb = const.tile([128, 128], f16)
            nc.scalar.dma_start(ident_sb[:], ident[:, :])

            # ---------------- local supports: sup'[r][n_loc, k] -------------
            # Relations are paired into one 512-wide moving operand (half the
            # matmuls); each direction's supports are emitted just before its
            # own stream so the first adjacency matmul starts early.
            sup_sb = const.tile([128, 2 * R, NT, K], f16)   # [p_n, r, nt, k]

            def emit_sup(dirn):
                for ri0, nr in ((0, 2), (2, 1)):            # pair + single
                    r0 = dirn * R + ri0
                    for nt in range(NT):
                        ps = psump.tile([128, nr * K], f32, tag="pb", name="psup")
                        for ht in range(HT):
                            nc.tensor.matmul(
                                ps[:],
                                inpsT_sb[:, ht, nt * 128 : (nt + 1) * 128],
                                wst_sb[:, r0 : r0 + nr, ht, :],
                                start=(ht == 0),
                                stop=(ht == HT - 1),
                            )
                        nc.vector.tensor_copy(sup_sb[:, r0 : r0 + nr, nt, :], ps[:])

            # ---------------- adjacency stream + RS staging ------------------
            stags = [
                dramp.tile([NC, 2, 128, NB], f16, name=f"stag{q}", tag=f"stag{q}")
                for q in range(2)
            ]
            rs_out = [
                dramp.tile([1, 2, 128, NB], f16, name=f"rs_out{q}", tag=f"rs_out{q}")
                for q in range(2)
            ]
            for dirn in range(2):                           # 0 = bw (h 0:256), 1 = fw
                emit_sup(dirn)
                for mc in range(NMC):
                    ps0 = psump.tile([128, MC], f32, tag="pb", name="ps0")  # k 0:128
                    ps1 = psump.tile([128, MC], f32, tag="pb", name="ps1")  # k 128:256
                    for ri in range(R):
                        r = dirn * R + ri
                        at = adjp.tile([128, NT, MC], adt, tag="adj")
                        nc.sync.dma_start(at[:], adjq[r, mc])
                        for nt in range(NT):
                            first = ri == 0 and nt == 0
                            last = ri == R - 1 and nt == NT - 1
                            for kk, ps in ((0, ps0), (1, ps1)):
                                lhsT = sup_sb[:, r, nt, kk * 128 : (kk + 1) * 128]
                                for mh in range(MC // 512):
                                    nc.tensor.matmul(
                                        ps[:, mh * 512 : (mh + 1) * 512],
                                        lhsT,
                                        at[:, nt, mh * 512 : (mh + 1) * 512],
                                        start=first,
                                        stop=last,
                                    )
                    for kk, ps in ((0, ps0), (1, ps1)):
                        ev = evacp.tile([128, MC], f16, tag="ev")
                        nc.vector.tensor_copy(ev[:], ps[:])
                        for d2 in range(MC // NB):
                            dest = (mc * MC) // NB + d2
                            nc.scalar.dma_start(
                                stags[dirn][dest, kk],
                                ev[:, d2 * NB : (d2 + 1) * NB],
                            )
                nc.gpsimd.collective_compute(
                    "ReduceScatter",
                    mybir.AluOpType.add,
                    replica_groups=[list(range(NC))],
                    ins=[stags[dirn][:].opt()],
                    outs=[rs_out[dirn][:].opt()],
                )

            # -------- bias + relu + final linear + residual ------------------
            # The final matmul accumulates per h-row block so each block's
            # matmuls run as soon as its direction's RS lands, overlapping the
            # remaining collective.
            frelu_sb = const.tile([128, HT, NB], f32r)      # [p_h, ht, m_loc]
            fts = []
            for dirn in range(2):                           # one load per RS result
                ft = evacp.tile([128, 2, NB], f16, tag="ftmp")
                nc.scalar.dma_start(
                    ft[:], rs_out[dirn][0].rearrange("k p n -> p k n")
                )
                fts.append(ft)
            psos = []
            for ht in range(HT):                            # ht -> (dir, k-half)
                nc.scalar.activation(
                    frelu_sb[:, ht, :],
                    fts[ht // 2][:, ht % 2, :],
                    Relu,
                    bias=biases_sb[:, ht, 0:1],
                )
                for jt in range(JT):
                    if ht == 0:
                        psos.append(
                            psump.tile([128, NB], f32, tag="pso", name=f"pso{jt}")
                        )
                    nc.tensor.matmul(
                        psos[jt][:],
                        w1_sb[:, ht, jt * 128 : (jt + 1) * 128],
                        frelu_sb[:, ht, :],
                        start=(ht == 0),
                        stop=False,
                    )
                    if ht == HT - 1:
                        # residual: psum += I.T @ inps^T block — keeps the
                        # tail chain off the vector engine entirely.
                        nc.tensor.matmul(
                            psos[jt][:],
                            ident_sb[:],
                            inpsT_sb[:, jt, :],
                            start=False,
                            stop=True,
                        )
            for jt in range(JT):
                ot = evacp.tile([128, NB], f16, tag="ev")
                nc.scalar.activation(
                    ot[:], psos[jt][:], Identity, bias=biases_sb[:, jt, 1:2]
                )
                nc.sync.dma_start(outT[:, jt, :], ot[:])

    nc.compile()
    nc.finalize()
    _BUILT["nc"] = nc
    return nc


def _round_fp32r(a):
    """Round fp32 to the fp32r (TF32-like, 1s+8e+11m in top 20 bits) format
    with round-to-nearest-even, as the PE's fp32r datapath expects."""
    b = np.ascontiguousarray(a, np.float32).view(np.uint32).astype(np.uint64)
    lsb = (b >> 12) & 1
    r = ((b + 0x7FF + lsb) & 0xFFFFF000).astype(np.uint32)
    return r.view(np.float32)


def _make_in_maps(inps, fw_adjs, bw_adjs, W_fw, b_fw, W_bw, b_bw, W1, b1):
    f = np.float32
    adt = ml_dtypes.float8_e3m4 if ADJ_MODE == "f8e3" else np.float16
    inps = np.asarray(inps, f)
    W1p = np.ascontiguousarray(
        _round_fp32r(np.asarray(W1, f)).reshape(HT, 128, H).transpose(1, 0, 2)
    )
    # W'/16 stacked [bw, fw], swizzled to [p, r, ht, k]
    wcat = np.concatenate([np.asarray(W_bw, f), np.asarray(W_fw, f)], axis=0)
    wst = np.ascontiguousarray(
        (wcat / ASCALE).reshape(2 * R, HT, 128, K).transpose(2, 0, 1, 3),
        np.float16,
    )
    # pre-relu bias: sum_r b_r + 0.5 * colsum(inps) @ sum_r W_r  (concat bw|fw)
    colsum = inps.sum(axis=0)                                 # [H]
    pre_bias = np.concatenate(
        [
            np.asarray(b_bw, f).sum(axis=0) + 0.5 * (colsum @ np.asarray(W_bw, f).sum(axis=0)),
            np.asarray(b_fw, f).sum(axis=0) + 0.5 * (colsum @ np.asarray(W_fw, f).sum(axis=0)),
        ]
    )                                                         # [H]
    biases = np.empty((128, JT, 2), f)
    biases[:, :, 0] = pre_bias.reshape(JT, 128).T
    biases[:, :, 1] = np.asarray(b1, f).reshape(JT, 128).T
    biases = np.ascontiguousarray(biases)

    fw_adjs = np.asarray(fw_adjs, f)
    bw_adjs = np.asarray(bw_adjs, f)

    in_maps = []
    for c in range(NC):
        sl = slice(c * NB, (c + 1) * NB)
        adjq_c = np.empty((2 * R, NMC, 128, NT, MC), adt)
        for dirn, adjs in ((0, bw_adjs), (1, fw_adjs)):
            for r in range(R):
                blk = adjs[r][:, sl].T                        # [n_loc, m]
                if ADJ_MODE == "f8e3":
                    blk = (blk - 0.5) * ASCALE
                # [nt, p, mc, m'] -> [mc, p, nt, m']
                adjq_c[dirn * R + r] = (
                    blk.reshape(NT, 128, NMC, MC).transpose(2, 1, 0, 3).astype(adt)
                )
        in_maps.append(
            {
                "inpsT": np.ascontiguousarray(
                    inps[sl].T.reshape(HT, 128, NB).transpose(1, 0, 2), np.float16
                ),
                "wst": wst,
                "adjq": adjq_c,
                "biases": biases,
                "w1": W1p,
                "ident": np.eye(128, dtype=np.float16),
            }
        )
    return in_maps


def run(trace=False, **inputs):
    """Run the SPMD kernel; returns (full_output, BassKernelResults)."""
    from concourse.bass_utils import run_bass_kernel_spmd

    nc = _build_nc()
    in_maps = _make_in_maps(**inputs)
    res = run_bass_kernel_spmd(nc, in_maps, core_ids=list(range(NC)), trace=trace)
    out = np.empty((N, H), np.float32)
    for c in range(NC):
        # outT [p, jt, n] -> out rows [n, h=jt*128+p]
        out[c * NB : (c + 1) * NB] = (
            res.results[c]["outT"].astype(np.float32).transpose(2, 1, 0).reshape(NB, H)
        )
    return out, res


def kernel(**inputs):
    # Collective-heavy SPMD runs have shown a rare corrupted execution
    # (launch-skew related). Executions are cheap next to compile, so run
    # twice and accept only agreeing results, with a third as tiebreaker.
    out1, _ = run(trace=False, **inputs)
    out2, _ = run(trace=False, **inputs)
    if np.array_equal(out1, out2):
        return out1
    out3, _ = run(trace=False, **inputs)
    return out3 if np.array_equal(out2, out3) else out1


# revision 15
# speedup vs baseline: 1.2377x; 1.2377x over previous
"""BiGCN layer kernel for 8 Trainium2 NeuronCores.

Hybrid-parallel SpMM with zero exposed collective time:
  - bw direction is 1D column-parallel: core c owns contraction slice
    n in [c*512, (c+1)*512) of the 3 bw adjacencies; partial feats^T
    [K, all m] accumulate in PSUM, stage to DRAM fp16, and ReduceScatter —
    triggered mid-kernel so the RS hides under the fw stream.
  - fw direction is 1D row-parallel: an AllGather of the six fw support
    slices (sup_fw = inps @ W/16, fp16, 0.75MB/rank) is triggered in the
    first ~20us and hides under the bw stream; each core then contracts its
    own m-block over ALL n, so the fw half of the output comes straight out
    of PSUM with no second collective on the critical tail.
  - Adjacency is stored as e3m4 fp8 of 16*(a - 0.5): the 0.5 shift centers
    uniform[0,1) values (halving quantization noise) and the x16 scale moves
    the payload into e3m4's normal range (subnormal-flush safe). The dropped
    mean term 0.5*sum_n sup[n,k] is a per-h constant the host folds into the
    pre-relu bias. PE matmuls mix fp16 stationary supports with fp8 moving
    adjacency.
  - All DRAM layouts are pre-swizzled on host so every DMA is 128 x
    contiguous lines. bias+relu fuse into scalar-engine activations (feats
    is produced transposed [h, m]); the final linear (fp32r) accumulates per
    h-row; the residual rides the same PSUM group via an identity matmul.
"""

import numpy as np
import ml_dtypes

N, H, R = 4096, 512, 3
K = H // 2            # 256
NC = 8                # cores
NB = N // NC          # 512 rows (m / n_loc) per core
MC = 1024             # bw m-chunk width per PSUM accumulation group
HT = H // 128         # 4 h-tiles
NT = NB // 128        # 4 n_loc tiles
JT = H // 128         # 4 output j tiles
NMC = N // MC         # 4 bw m chunks
ASCALE = 16.0         # adjacency fp8 scale: q = e3m4(16*(a-0.5))

# adjacency on-chip dtype: "f8e3" (half DMA, mixed-dtype matmul) or "f16"
ADJ_MODE = "f8e3"

_BUILT = {}


def _build_nc():
    """Build (and cache) the Bass program. Identical program on all 8 cores."""
    if "nc" in _BUILT:
        return _BUILT["nc"]

    import concourse.bass as bass
    import concourse.mybir as mybir
    from concourse import bacc, tile

    f32 = mybir.dt.float32
    f32r = mybir.dt.float32r
    f16 = mybir.dt.float16
    adt = mybir.dt.float8e3 if ADJ_MODE == "f8e3" else f16
    nc = bacc.Bacc(None, num_devices=NC)

    inpsT = nc.dram_tensor("inpsT", [128, HT, NB], f16, kind="ExternalInput")
    wst = nc.dram_tensor("wst", [128, 2 * R, HT, K], f16, kind="ExternalInput")
    adjbw = nc.dram_tensor("adjbw", [R, NMC, 128, NT, MC], adt, kind="ExternalInput")
    adjfw = nc.dram_tensor("adjfw", [R, NC, 128, NT, NB], adt, kind="ExternalInput")
    biases = nc.dram_tensor("biases", [128, JT, 2], f32, kind="ExternalInput")
    w1 = nc.dram_tensor("w1", [128, HT, H], f32r, kind="ExternalInput")
    ident = nc.dram_tensor("ident", [128, 128], f16, kind="ExternalInput")
    outT = nc.dram_tensor("outT", [128, JT, NB], f16, kind="ExternalOutput")

    Relu = mybir.ActivationFunctionType.Relu
    Identity = mybir.ActivationFunctionType.Identity

    with tile.TileContext(nc) as tc:
        with (
            tc.tile_pool(name="const", bufs=1) as const,
            tc.tile_pool(name="adjp", bufs=9) as adjp,
            tc.tile_pool(name="supag", bufs=3) as supagp,
            tc.tile_pool(name="evacp", bufs=3) as evacp,
            tc.tile_pool(name="psum", bufs=2, space=bass.MemorySpace.PSUM) as psump,
            tc.tile_pool(name="psum2", bufs=4, space=bass.MemorySpace.PSUM) as psum2,
            tc.tile_pool(name="dram", bufs=1, space="DRAM") as dramp,
        ):
            # ---------------- constants into SBUF (all pre-swizzled) --------
            # fw weights first: sup_fw feeds the AllGather, so it is emitted
            # before anything else. Epilogue constants ride the scalar ring.
            inpsT_sb = const.tile([128, HT, NB], f16)       # [p_h, ht, n_loc]
            wst_sb = const.tile([128, 2 * R, HT, K], f16)   # [p_h, r, ht, k]
            for ht in range(HT):
                nc.sync.dma_start(inpsT_sb[:, ht], inpsT[:, ht])
                nc.sync.dma_start(wst_sb[:, R : 2 * R, ht], wst[:, R : 2 * R, ht])
            nc.sync.dma_start(wst_sb[:, 0:R], wst[:, 0:R])
            biases_sb = const.tile([128, JT, 2], f32)
            nc.scalar.dma_start(biases_sb[:], biases[:, :, :])
            w1_sb = const.tile([128, HT, H], f32r)          # [p_h, ht, j]
            nc.scalar.dma_start(w1_sb[:], w1[:, :, :])
            ident_sb = const.tile([128, 128], f16)
            nc.scalar.dma_start(ident_sb[:], ident[:, :])

            # ---------------- local supports: sup'[r][n_loc, k] -------------
            sup_sb = const.tile([128, 2 * R, NT, K], f16)   # [p_n, r, nt, k]

            def emit_sup(dirn):
                for ri0, nr in ((0, 2), (2, 1)):            # pair + single
                    r0 = dirn * R + ri0
                    for nt in range(NT):
                        ps = psump.tile([128, nr * K], f32, tag="pb", name="psup")
                        for ht in range(HT):
                            nc.tensor.matmul(
                                ps[:],
                                inpsT_sb[:, ht, nt * 128 : (nt + 1) * 128],
                                wst_sb[:, r0 : r0 + nr, ht, :],
                                start=(ht == 0),
                                stop=(ht == HT - 1),
                            )
                        nc.vector.tensor_copy(sup_sb[:, r0 : r0 + nr, nt, :], ps[:])

            # fw supports first -> AllGather them across cores ASAP.
            emit_sup(1)
            ag_in = dramp.tile([128, R, NT, K], f16, name="ag_in", tag="ag_in")
            ag_out = dramp.tile([NC, 128, R, NT, K], f16, name="ag_out", tag="ag_out")
            nc.scalar.dma_start(ag_in[:], sup_sb[:, R : 2 * R])
            nc.gpsimd.collective_compute(
                "AllGather",
                mybir.AluOpType.bypass,
                replica_groups=[list(range(NC))],
                ins=[ag_in[:].opt()],
                outs=[ag_out[:].opt()],
            )
            emit_sup(0)

            # ---------------- bw stream (column-parallel, kk-major) ----------
            stag = dramp.tile([NC, 2, 128, NB], f16, name="stag", tag="stag")
            rs_out = dramp.tile([1, 2, 128, NB], f16, name="rs_out", tag="rs_out")
            for mc in range(NMC):
                ats = []
                for ri in range(R):
                    at = adjp.tile([128, NT, MC], adt, tag="adj")
                    nc.sync.dma_start(at[:], adjbw[ri, mc])
                    ats.append(at)
                for kk in range(2):                         # k-half major: ps0
                    ps = psump.tile([128, MC], f32, tag="pb", name=f"ps{kk}")
                    for ri in range(R):                     # evacuates while the
                        for nt in range(NT):                # other half computes
                            lhsT = sup_sb[:, ri, nt, kk * 128 : (kk + 1) * 128]
                            for mh in range(MC // 512):
                                nc.tensor.matmul(
                                    ps[:, mh * 512 : (mh + 1) * 512],
                                    lhsT,
                                    ats[ri][:, nt, mh * 512 : (mh + 1) * 512],
                                    start=(ri == 0 and nt == 0),
                                    stop=(ri == R - 1 and nt == NT - 1),
                                )
                    ev = evacp.tile([128, MC], f16, tag="ev")
                    nc.vector.tensor_copy(ev[:], ps[:])
                    for d2 in range(MC // NB):
                        dest = (mc * MC) // NB + d2
                        nc.scalar.dma_start(
                            stag[dest, kk], ev[:, d2 * NB : (d2 + 1) * NB]
                        )
            nc.gpsimd.collective_compute(
                "ReduceScatter",
                mybir.AluOpType.add,
                replica_groups=[list(range(NC))],
                ins=[stag[:].opt()],
                outs=[rs_out[:].opt()],
            )

            # ---------------- fw stream (row-parallel over own m-block) ------
            psf = [
                psump.tile([128, NB], f32, tag="pb", name=f"psf{kk}")
                for kk in range(2)
            ]
            for c2 in range(NC):
                sg = supagp.tile([128, R, NT, K], f16, tag="sg")
                nc.sync.dma_start(sg[:], ag_out[c2])
                for ri in range(R):
                    at = adjp.tile([128, NT, NB], adt, tag="adjf")
                    nc.sync.dma_start(at[:], adjfw[ri, c2])
                    for nt in range(NT):
                        for kk in range(2):
                            nc.tensor.matmul(
                                psf[kk][:],
                                sg[:, ri, nt, kk * 128 : (kk + 1) * 128],
                                at[:, nt, :],
                                start=(c2 == 0 and ri == 0 and nt == 0),
                                stop=(c2 == NC - 1 and ri == R - 1 and nt == NT - 1),
                            )

            # -------- bias + relu + final linear + residual ------------------
            # bw halves (ht 0,1) come from the RS result and can run during
            # the fw stream; fw halves (ht 2,3) read straight from PSUM.
            frelu_sb = const.tile([128, HT, NB], f32r)      # [p_h, ht, m_loc]
            ft = evacp.tile([128, 2, NB], f16, tag="ftmp")
            nc.scalar.dma_start(ft[:], rs_out[0].rearrange("k p n -> p k n"))
            psos = [
                psum2.tile([128, NB], f32, tag="pso", name=f"pso{jt}")
                for jt in range(JT)
            ]
            for ht in range(HT):                            # ht -> (dir, k-half)
                if ht < 2:
                    nc.scalar.activation(
                        frelu_sb[:, ht, :],
                        ft[:, ht, :],
                        Relu,
                        bias=biases_sb[:, ht, 0:1],
                    )
                else:
                    nc.scalar.activation(
                        frelu_sb[:, ht, :],
                        psf[ht - 2][:],
                        Relu,
                        bias=biases_sb[:, ht, 0:1],
                    )
                for jt in range(JT):
                    nc.tensor.matmul(
                        psos[jt][:],
                        w1_sb[:, ht, jt * 128 : (jt + 1) * 128],
                        frelu_sb[:, ht, :],
                        start=(ht == 0),
                        stop=False,
                    )
                    if ht == HT - 1:
                        # residual: psum += I.T @ inps^T block
                        nc.tensor.matmul(
                            psos[jt][:],
                            ident_sb[:],
                            inpsT_sb[:, jt, :],
                            start=False,
                            stop=True,
                        )
            for jt in range(JT):
                ot = evacp.tile([128, NB], f16, tag="ev")
                nc.scalar.activation(
                    ot[:], psos[jt][:], Identity, bias=biases_sb[:, jt, 1:2]
                )
                nc.sync.dma_start(outT[:, jt, :], ot[:])

    nc.compile()
    nc.finalize()
    _BUILT["nc"] = nc
    return nc


def _round_fp32r(a):
    """Round fp32 to the fp32r (TF32-like, 1s+8e+11m in top 20 bits) format
    with round-to-nearest-even, as the PE's fp32r datapath expects."""
    b = np.ascontiguousarray(a, np.float32).view(np.uint32).astype(np.uint64)
    lsb = (b >> 12) & 1
    r = ((b + 0x7FF + lsb) & 0xFFFFF000).astype(np.uint32)
    return r.view(np.float32)


def _make_in_maps(inps, fw_adjs, bw_adjs, W_fw, b_fw, W_bw, b_bw, W1, b1):
    f = np.float32
    adt = ml_dtypes.float8_e3m4 if ADJ_MODE == "f8e3" else np.float16
    inps = np.asarray(inps, f)
    W1p = np.ascontiguousarray(
        _round_fp32r(np.asarray(W1, f)).reshape(HT, 128, H).transpose(1, 0, 2)
    )
    # W'/16 stacked [bw, fw], swizzled to [p, r, ht, k]
    wcat = np.concatenate([np.asarray(W_bw, f), np.asarray(W_fw, f)], axis=0)
    wst = np.ascontiguousarray(
        (wcat / ASCALE).reshape(2 * R, HT, 128, K).transpose(2, 0, 1, 3),
        np.float16,
    )
    # pre-relu bias: sum_r b_r + 0.5 * colsum(inps) @ sum_r W_r  (concat bw|fw)
    colsum = inps.sum(axis=0)                                 # [H]
    pre_bias = np.concatenate(
        [
            np.asarray(b_bw, f).sum(axis=0) + 0.5 * (colsum @ np.asarray(W_bw, f).sum(axis=0)),
            np.asarray(b_fw, f).sum(axis=0) + 0.5 * (colsum @ np.asarray(W_fw, f).sum(axis=0)),
        ]
    )                                                         # [H]
    biases = np.empty((128, JT, 2), f)
    biases[:, :, 0] = pre_bias.reshape(JT, 128).T
    biases[:, :, 1] = np.asarray(b1, f).reshape(JT, 128).T
    biases = np.ascontiguousarray(biases)

    fw_adjs = np.asarray(fw_adjs, f)
    bw_adjs = np.asarray(bw_adjs, f)

    def q(blk):
        if ADJ_MODE == "f8e3":
            blk = (blk - 0.5) * ASCALE
        return blk.astype(adt)

    in_maps = []
    for c in range(NC):
        sl = slice(c * NB, (c + 1) * NB)
        # bw: column-parallel — contraction slice n in sl, all m.
        adjbw_c = np.empty((R, NMC, 128, NT, MC), adt)
        for r in range(R):
            blk = q(bw_adjs[r][:, sl].T)                      # [n_loc, m]
            adjbw_c[r] = blk.reshape(NT, 128, NMC, MC).transpose(2, 1, 0, 3)
        # fw: row-parallel — own m rows in sl, all n (transposed).
        adjfw_c = np.empty((R, NC, 128, NT, NB), adt)
        for r in range(R):
            blk = q(fw_adjs[r][sl, :].T)                      # [n_all, m_own]
            adjfw_c[r] = blk.reshape(NC, NT, 128, NB).transpose(0, 2, 1, 3)
        in_maps.append(
            {
                "inpsT": np.ascontiguousarray(
                    inps[sl].T.reshape(HT, 128, NB).transpose(1, 0, 2), np.float16
                ),
                "wst": wst,
                "adjbw": adjbw_c,
                "adjfw": adjfw_c,
                "biases": biases,
                "w1": W1p,
                "ident": np.eye(128, dtype=np.float16),
            }
        )
    return in_maps


def run(trace=False, **inputs):
    """Run the SPMD kernel; returns (full_output, BassKernelResults)."""
    from concourse.bass_utils import run_bass_kernel_spmd

    nc = _build_nc()
    in_maps = _make_in_maps(**inputs)
    res = run_bass_kernel_spmd(nc, in_maps, core_ids=list(range(NC)), trace=trace)
    out = np.empty((N, H), np.float32)
    for c in range(NC):
        # outT [p, jt, n] -> out rows [n, h=jt*128+p]
        out[c * NB : (c + 1) * NB] = (
            res.results[c]["outT"].astype(np.float32).transpose(2, 1, 0).reshape(NB, H)
        )
    return out, res


def kernel(**inputs):
    # Collective-heavy SPMD runs have shown a rare corrupted execution
    # (launch-skew related). Executions are cheap next to compile, so run
    # twice and accept only agreeing results, with a third as tiebreaker.
    out1, _ = run(trace=False, **inputs)
    out2, _ = run(trace=False, **inputs)
    if np.array_equal(out1, out2):
        return out1
    out3, _ = run(trace=False, **inputs)
    return out3 if np.array_equal(out2, out3) else out1


# revision 21
# speedup vs baseline: 1.2769x; 1.0317x over previous
"""BiGCN layer kernel for 8 Trainium2 NeuronCores.

Hybrid-parallel SpMM with zero exposed collective time:
  - bw direction is 1D column-parallel: core c owns contraction slice
    n in [c*512, (c+1)*512) of the 3 bw adjacencies; partial feats^T
    [K, all m] accumulate in PSUM, stage to DRAM fp16, and ReduceScatter —
    triggered mid-kernel so the RS hides under the fw stream.
  - fw direction is 1D row-parallel: an AllGather of the six fw support
    slices (sup_fw = inps @ W/16, fp16, 0.75MB/rank) is triggered in the
    first ~20us and hides under the bw stream; each core then contracts its
    own m-block over ALL n, so the fw half of the output comes straight out
    of PSUM with no second collective on the critical tail.
  - Adjacency is stored as e3m4 fp8 of 16*(a - 0.5): the 0.5 shift centers
    uniform[0,1) values (halving quantization noise) and the x16 scale moves
    the payload into e3m4's normal range (subnormal-flush safe). The dropped
    mean term 0.5*sum_n sup[n,k] is a per-h constant the host folds into the
    pre-relu bias. PE matmuls mix fp16 stationary supports with fp8 moving
    adjacency.
  - All DRAM layouts are pre-swizzled on host so every DMA is 128 x
    contiguous lines. bias+relu fuse into scalar-engine activations (feats
    is produced transposed [h, m]); the final linear (fp32r) accumulates per
    h-row; the residual rides the same PSUM group via an identity matmul.
"""

import numpy as np
import ml_dtypes

N, H, R = 4096, 512, 3
K = H // 2            # 256
NC = 8                # cores
NB = N // NC          # 512 rows (m / n_loc) per core
MC = 1024             # bw m-chunk width per PSUM accumulation group
HT = H // 128         # 4 h-tiles
NT = NB // 128        # 4 n_loc tiles
JT = H // 128         # 4 output j tiles
NMC = N // MC         # 4 bw m chunks
ASCALE = 16.0         # adjacency fp8 scale: q = e3m4(16*(a-0.5))

# adjacency on-chip dtype: "f8e3" (half DMA, mixed-dtype matmul) or "f16"
ADJ_MODE = "f8e3"

_BUILT = {}


def _build_nc():
    """Build (and cache) the Bass program. Identical program on all 8 cores."""
    if "nc" in _BUILT:
        return _BUILT["nc"]

    import concourse.bass as bass
    import concourse.mybir as mybir
    from concourse import bacc, tile

    f32 = mybir.dt.float32
    f32r = mybir.dt.float32r
    f16 = mybir.dt.float16
    adt = mybir.dt.float8e3 if ADJ_MODE == "f8e3" else f16
    nc = bacc.Bacc(None, num_devices=NC)

    inpsT = nc.dram_tensor("inpsT", [128, HT, NB], f16, kind="ExternalInput")
    wst = nc.dram_tensor("wst", [128, 2 * R, HT, K], f16, kind="ExternalInput")
    adjbw = nc.dram_tensor("adjbw", [R, NMC, 128, NT, MC], adt, kind="ExternalInput")
    adjfw = nc.dram_tensor("adjfw", [R, NC, 128, NT, NB], adt, kind="ExternalInput")
    biases = nc.dram_tensor("biases", [128, JT, 2], f32, kind="ExternalInput")
    w1 = nc.dram_tensor("w1", [128, HT, H], f32r, kind="ExternalInput")
    ident = nc.dram_tensor("ident", [128, 128], f16, kind="ExternalInput")
    outT = nc.dram_tensor("outT", [128, JT, NB], f16, kind="ExternalOutput")

    Relu = mybir.ActivationFunctionType.Relu
    Identity = mybir.ActivationFunctionType.Identity

    with tile.TileContext(nc) as tc:
        with (
            tc.tile_pool(name="const", bufs=1) as const,
            tc.tile_pool(name="adjp", bufs=9) as adjp,
            tc.tile_pool(name="adjfp", bufs=24) as adjfp,
            tc.tile_pool(name="supag", bufs=3) as supagp,
            tc.tile_pool(name="evacp", bufs=3) as evacp,
            tc.tile_pool(name="psum", bufs=2, space=bass.MemorySpace.PSUM) as psump,
            tc.tile_pool(name="psum2", bufs=4, space=bass.MemorySpace.PSUM) as psum2,
            tc.tile_pool(name="dram", bufs=1, space="DRAM") as dramp,
        ):
            # ---------------- constants into SBUF (all pre-swizzled) --------
            # fw weights first: sup_fw feeds the AllGather, so it is emitted
            # before anything else. Epilogue constants ride the scalar ring.
            inpsT_sb = const.tile([128, HT, NB], f16)       # [p_h, ht, n_loc]
            wst_sb = const.tile([128, 2 * R, HT, K], f16)   # [p_h, r, ht, k]
            for ht in range(HT):
                nc.sync.dma_start(inpsT_sb[:, ht], inpsT[:, ht])
                nc.sync.dma_start(wst_sb[:, R : 2 * R, ht], wst[:, R : 2 * R, ht])
            nc.sync.dma_start(wst_sb[:, 0:R], wst[:, 0:R])
            biases_sb = const.tile([128, JT, 2], f32)
            w1_sb = const.tile([128, HT, H], f32r)          # [p_h, ht, j]
            ident_sb = const.tile([128, 128], f16)

            # ---------------- local supports: sup'[r][n_loc, k] -------------
            sup_sb = const.tile([128, 2 * R, NT, K], f16)   # [p_n, r, nt, k]

            def emit_sup(dirn):
                for ri0, nr in ((0, 2), (2, 1)):            # pair + single
                    r0 = dirn * R + ri0
                    for nt in range(NT):
                        ps = psump.tile([128, nr * K], f32, tag="pb", name="psup")
                        for ht in range(HT):
                            nc.tensor.matmul(
                                ps[:],
                                inpsT_sb[:, ht, nt * 128 : (nt + 1) * 128],
                                wst_sb[:, r0 : r0 + nr, ht, :],
                                start=(ht == 0),
                                stop=(ht == HT - 1),
                            )
                        nc.vector.tensor_copy(sup_sb[:, r0 : r0 + nr, nt, :], ps[:])

            # fw supports first -> AllGather them across cores ASAP. The
            # ag_in stage is the scalar ring's first DMA so the collective's
            # data dependency clears as early as possible.
            emit_sup(1)
            ag_in = dramp.tile([128, R, NT, K], f16, name="ag_in", tag="ag_in")
            ag_out = dramp.tile([NC, 128, R, NT, K], f16, name="ag_out", tag="ag_out")
            nc.scalar.dma_start(ag_in[:], sup_sb[:, R : 2 * R])
            nc.gpsimd.collective_compute(
                "AllGather",
                mybir.AluOpType.bypass,
                replica_groups=[list(range(NC))],
                ins=[ag_in[:].opt()],
                outs=[ag_out[:].opt()],
            )
            nc.scalar.dma_start(biases_sb[:], biases[:, :, :])
            nc.scalar.dma_start(w1_sb[:], w1[:, :, :])
            nc.scalar.dma_start(ident_sb[:], ident[:, :])
            emit_sup(0)

            # ---------------- bw stream (column-parallel, kk-major) ----------
            stag = dramp.tile([NC, 2, 128, NB], f16, name="stag", tag="stag")
            rs_out = dramp.tile([1, 2, 128, NB], f16, name="rs_out", tag="rs_out")
            for mc in range(NMC):
                ats = []
                for ri in range(R):
                    at = adjp.tile([128, NT, MC], adt, tag="adj")
                    nc.sync.dma_start(at[:], adjbw[ri, mc])
                    ats.append(at)
                for kk in range(2):                         # k-half major: ps0
                    ps = psump.tile([128, MC], f32, tag="pb", name=f"ps{kk}")
                    for ri in range(R):                     # evacuates while the
                        for nt in range(NT):                # other half computes
                            lhsT = sup_sb[:, ri, nt, kk * 128 : (kk + 1) * 128]
                            for mh in range(MC // 512):
                                nc.tensor.matmul(
                                    ps[:, mh * 512 : (mh + 1) * 512],
                                    lhsT,
                                    ats[ri][:, nt, mh * 512 : (mh + 1) * 512],
                                    start=(ri == 0 and nt == 0),
                                    stop=(ri == R - 1 and nt == NT - 1),
                                )
                    ev = evacp.tile([128, MC], f16, tag="ev")
                    nc.vector.tensor_copy(ev[:], ps[:])
                    for d2 in range(MC // NB):
                        dest = (mc * MC) // NB + d2
                        nc.scalar.dma_start(
                            stag[dest, kk], ev[:, d2 * NB : (d2 + 1) * NB]
                        )
            nc.gpsimd.collective_compute(
                "ReduceScatter",
                mybir.AluOpType.add,
                replica_groups=[list(range(NC))],
                ins=[stag[:].opt()],
                outs=[rs_out[:].opt()],
            )

            # ---------------- fw stream (row-parallel over own m-block) ------
            psf = [
                psump.tile([128, NB], f32, tag="pb", name=f"psf{kk}")
                for kk in range(2)
            ]
            for c2 in range(NC):
                # sg rides the scalar ring: its AllGather-gated semaphore wait
                # must not stall the sync ring's adjacency prefetch behind it.
                sg = supagp.tile([128, R, NT, K], f16, tag="sg")
                nc.scalar.dma_start(sg[:], ag_out[c2])
                for ri in range(R):
                    at = adjfp.tile([128, NT, NB], adt, tag="adjf")
                    nc.sync.dma_start(at[:], adjfw[ri, c2])
                    for nt in range(NT):
                        for kk in range(2):
                            nc.tensor.matmul(
                                psf[kk][:],
                                sg[:, ri, nt, kk * 128 : (kk + 1) * 128],
                                at[:, nt, :],
                                start=(c2 == 0 and ri == 0 and nt == 0),
                                stop=(c2 == NC - 1 and ri == R - 1 and nt == NT - 1),
                            )

            # -------- bias + relu + final linear + residual ------------------
            # bw halves (ht 0,1) come from the RS result and can run during
            # the fw stream; fw halves (ht 2,3) read straight from PSUM.
            frelu_sb = const.tile([128, HT, NB], f32r)      # [p_h, ht, m_loc]
            ft = evacp.tile([128, 2, NB], f16, tag="ftmp")
            nc.scalar.dma_start(ft[:], rs_out[0].rearrange("k p n -> p k n"))
            psos = [
                psum2.tile([128, NB], f32, tag="pso", name=f"pso{jt}")
                for jt in range(JT)
            ]
            for ht in range(HT):                            # ht -> (dir, k-half)
                if ht < 2:
                    nc.scalar.activation(
                        frelu_sb[:, ht, :],
                        ft[:, ht, :],
                        Relu,
                        bias=biases_sb[:, ht, 0:1],
                    )
                else:
                    nc.scalar.activation(
                        frelu_sb[:, ht, :],
                        psf[ht - 2][:],
                        Relu,
                        bias=biases_sb[:, ht, 0:1],
                    )
                for jt in range(JT):
                    nc.tensor.matmul(
                        psos[jt][:],
                        w1_sb[:, ht, jt * 128 : (jt + 1) * 128],
                        frelu_sb[:, ht, :],
                        start=(ht == 0),
                        stop=False,
                    )
                    if ht == HT - 1:
                        # residual: psum += I.T @ inps^T block
                        nc.tensor.matmul(
                            psos[jt][:],
                            ident_sb[:],
                            inpsT_sb[:, jt, :],
                            start=False,
                            stop=True,
                        )
            for jt in range(JT):
                ot = evacp.tile([128, NB], f16, tag="ev")
                nc.scalar.activation(
                    ot[:], psos[jt][:], Identity, bias=biases_sb[:, jt, 1:2]
                )
                nc.sync.dma_start(outT[:, jt, :], ot[:])

    nc.compile()
    nc.finalize()
    _BUILT["nc"] = nc
    return nc


def _round_fp32r(a):
    """Round fp32 to the fp32r (TF32-like, 1s+8e+11m in top 20 bits) format
    with round-to-nearest-even, as the PE's fp32r datapath expects."""
    b = np.ascontiguousarray(a, np.float32).view(np.uint32).astype(np.uint64)
    lsb = (b >> 12) & 1
    r = ((b + 0x7FF + lsb) & 0xFFFFF000).astype(np.uint32)
    return r.view(np.float32)


def _make_in_maps(inps, fw_adjs, bw_adjs, W_fw, b_fw, W_bw, b_bw, W1, b1):
    f = np.float32
    adt = ml_dtypes.float8_e3m4 if ADJ_MODE == "f8e3" else np.float16
    inps = np.asarray(inps, f)
    W1p = np.ascontiguousarray(
        _round_fp32r(np.asarray(W1, f)).reshape(HT, 128, H).transpose(1, 0, 2)
    )
    # W'/16 stacked [bw, fw], swizzled to [p, r, ht, k]
    wcat = np.concatenate([np.asarray(W_bw, f), np.asarray(W_fw, f)], axis=0)
    wst = np.ascontiguousarray(
        (wcat / ASCALE).reshape(2 * R, HT, 128, K).transpose(2, 0, 1, 3),
        np.float16,
    )
    # pre-relu bias: sum_r b_r + 0.5 * colsum(inps) @ sum_r W_r  (concat bw|fw)
    colsum = inps.sum(axis=0)                                 # [H]
    pre_bias = np.concatenate(
        [
            np.asarray(b_bw, f).sum(axis=0) + 0.5 * (colsum @ np.asarray(W_bw, f).sum(axis=0)),
            np.asarray(b_fw, f).sum(axis=0) + 0.5 * (colsum @ np.asarray(W_fw, f).sum(axis=0)),
        ]
    )                                                         # [H]
    biases = np.empty((128, JT, 2), f)
    biases[:, :, 0] = pre_bias.reshape(JT, 128).T
    biases[:, :, 1] = np.asarray(b1, f).reshape(JT, 128).T
    biases = np.ascontiguousarray(biases)

    fw_adjs = np.asarray(fw_adjs, f)
    bw_adjs = np.asarray(bw_adjs, f)

    def q(blk):
        if ADJ_MODE == "f8e3":
            blk = (blk - 0.5) * ASCALE
        return blk.astype(adt)

    in_maps = []
    for c in range(NC):
        sl = slice(c * NB, (c + 1) * NB)
        # bw: column-parallel — contraction slice n in sl, all m.
        adjbw_c = np.empty((R, NMC, 128, NT, MC), adt)
        for r in range(R):
            blk = q(bw_adjs[r][:, sl].T)                      # [n_loc, m]
            adjbw_c[r] = blk.reshape(NT, 128, NMC, MC).transpose(2, 1, 0, 3)
        # fw: row-parallel — own m rows in sl, all n (transposed).
        adjfw_c = np.empty((R, NC, 128, NT, NB), adt)
        for r in range(R):
            blk = q(fw_adjs[r][sl, :].T)                      # [n_all, m_own]
            adjfw_c[r] = blk.reshape(NC, NT, 128, NB).transpose(0, 2, 1, 3)
        in_maps.append(
            {
                "inpsT": np.ascontiguousarray(
                    inps[sl].T.reshape(HT, 128, NB).transpose(1, 0, 2), np.float16
                ),
                "wst": wst,
                "adjbw": adjbw_c,
                "adjfw": adjfw_c,
                "biases": biases,
                "w1": W1p,
                "ident": np.eye(128, dtype=np.float16),
            }
        )
    return in_maps


def run(trace=False, **inputs):
    """Run the SPMD kernel; returns (full_output, BassKernelResults)."""
    from concourse.bass_utils import run_bass_kernel_spmd

    nc = _build_nc()
    in_maps = _make_in_maps(**inputs)
    res = run_bass_kernel_spmd(nc, in_maps, core_ids=list(range(NC)), trace=trace)
    out = np.empty((N, H), np.float32)
    for c in range(NC):
        # outT [p, jt, n] -> out rows [n, h=jt*128+p]
        out[c * NB : (c + 1) * NB] = (
            res.results[c]["outT"].astype(np.float32).transpose(2, 1, 0).reshape(NB, H)
        )
    return out, res


def kernel(**inputs):
    # Collective-heavy SPMD runs have shown a rare corrupted execution
    # (launch-skew related). Executions are cheap next to compile, so run
    # twice and accept only agreeing results, with a third as tiebreaker.
    out1, _ = run(trace=False, **inputs)
    out2, _ = run(trace=False, **inputs)
    if np.array_equal(out1, out2):
        return out1
    out3, _ = run(trace=False, **inputs)
    return out3 if np.array_equal(out2, out3) else out1


# revision 26
# speedup vs baseline: 1.4377x; 1.1260x over previous
"""BiGCN layer kernel for 8 Trainium2 NeuronCores.

Hybrid-parallel SpMM with zero exposed collective time:
  - bw direction is 1D column-parallel: core c owns contraction slice
    n in [c*512, (c+1)*512) of the 3 bw adjacencies; partial feats^T
    [K, all m] accumulate in PSUM, stage to DRAM fp16, and ReduceScatter —
    triggered mid-kernel so the RS hides under the fw stream.
  - fw direction is 1D row-parallel: an AllGather of the six fw support
    slices (sup_fw = inps @ W/16, fp16, 0.75MB/rank) is triggered in the
    first ~20us and hides under the bw stream; each core then contracts its
    own m-block over ALL n, so the fw half of the output comes straight out
    of PSUM with no second collective on the critical tail.
  - Adjacency is stored as e3m4 fp8 of 16*(a - 0.5): the 0.5 shift centers
    uniform[0,1) values (halving quantization noise) and the x16 scale moves
    the payload into e3m4's normal range (subnormal-flush safe). The dropped
    mean term 0.5*sum_n sup[n,k] is a per-h constant the host folds into the
    pre-relu bias. PE matmuls mix fp16 stationary supports with fp8 moving
    adjacency.
  - All DRAM layouts are pre-swizzled on host so every DMA is 128 x
    contiguous lines. bias+relu fuse into scalar-engine activations (feats
    is produced transposed [h, m]); the final linear (fp32r) accumulates per
    h-row; the residual rides the same PSUM group via an identity matmul.
"""

import numpy as np
import ml_dtypes

N, H, R = 4096, 512, 3
K = H // 2            # 256
NC = 8                # cores
NB = N // NC          # 512 rows (m / n_loc) per core
MC = 1024             # bw m-chunk width per PSUM accumulation group
HT = H // 128         # 4 h-tiles
NT = NB // 128        # 4 n_loc tiles
JT = H // 128         # 4 output j tiles
NMC = N // MC         # 4 bw m chunks
ASCALE = 16.0         # adjacency fp8 scale: q = e3m4(16*(a-0.5))

# adjacency on-chip dtype: "f8e3" (half DMA, mixed-dtype matmul) or "f16"
ADJ_MODE = "f8e3"

_BUILT = {}


def _build_nc():
    """Build (and cache) the Bass program. Identical program on all 8 cores."""
    if "nc" in _BUILT:
        return _BUILT["nc"]

    import concourse.bass as bass
    import concourse.mybir as mybir
    from concourse import bacc, tile

    f32 = mybir.dt.float32
    f32r = mybir.dt.float32r
    f16 = mybir.dt.float16
    adt = mybir.dt.float8e3 if ADJ_MODE == "f8e3" else f16
    nc = bacc.Bacc(None, num_devices=NC)

    inpsT = nc.dram_tensor("inpsT", [128, HT, NB], f16, kind="ExternalInput")
    wst = nc.dram_tensor("wst", [128, 2 * R, HT, K], f16, kind="ExternalInput")
    adjbw = nc.dram_tensor("adjbw", [R, NMC, 128, NT, MC], adt, kind="ExternalInput")
    adjfw = nc.dram_tensor("adjfw", [R, NC, 128, NT, NB], adt, kind="ExternalInput")
    biases = nc.dram_tensor("biases", [128, JT, 2], f32, kind="ExternalInput")
    w1 = nc.dram_tensor("w1", [128, HT, H], f32r, kind="ExternalInput")
    ident = nc.dram_tensor("ident", [128, 128], f16, kind="ExternalInput")
    outT = nc.dram_tensor("outT", [128, JT, NB], f16, kind="ExternalOutput")

    Relu = mybir.ActivationFunctionType.Relu
    Identity = mybir.ActivationFunctionType.Identity

    with tile.TileContext(nc) as tc:
        with (
            tc.tile_pool(name="const", bufs=1) as const,
            tc.tile_pool(name="adjp", bufs=9) as adjp,
            tc.tile_pool(name="adjfp", bufs=24) as adjfp,
            tc.tile_pool(name="supag", bufs=3) as supagp,
            tc.tile_pool(name="evacp", bufs=3) as evacp,
            tc.tile_pool(name="psum", bufs=2, space=bass.MemorySpace.PSUM) as psump,
            tc.tile_pool(name="psum2", bufs=4, space=bass.MemorySpace.PSUM) as psum2,
            tc.tile_pool(name="dram", bufs=1, space="DRAM") as dramp,
        ):
            # ---------------- constants into SBUF (all pre-swizzled) --------
            # fw weights first: sup_fw feeds the AllGather, so it is emitted
            # before anything else. Epilogue constants ride the scalar ring.
            inpsT_sb = const.tile([128, HT, NB], f16)       # [p_h, ht, n_loc]
            wst_sb = const.tile([128, 2 * R, HT, K], f16)   # [p_h, r, ht, k]
            for ht in range(HT):
                nc.sync.dma_start(inpsT_sb[:, ht], inpsT[:, ht])
                nc.sync.dma_start(wst_sb[:, R : 2 * R, ht], wst[:, R : 2 * R, ht])
            nc.sync.dma_start(wst_sb[:, 0:R], wst[:, 0:R])
            biases_sb = const.tile([128, JT, 2], f32)
            w1_sb = const.tile([128, HT, H], f32r)          # [p_h, ht, j]
            ident_sb = const.tile([128, 128], f16)

            # ---------------- local supports: sup'[r][n_loc, k] -------------
            # bw supports live in fp16 (weights pre-scaled by 1/16); fw
            # supports are cast to e3m4 (weights unscaled, values ~N(0,0.6)
            # sit in fp8's normal range) to halve the AllGather payload. The
            # 16x product domain is undone by the fw relu's scale=1/16.
            sup_sb = const.tile([128, R, NT, K], f16)       # bw  [p_n, r, nt, k]
            supq_sb = const.tile([128, R, NT, K], adt)      # fw  [p_n, r, nt, k]

            def emit_sup(dirn, dst):
                for ri0, nr in ((0, 2), (2, 1)):            # pair + single
                    for nt in range(NT):
                        ps = psump.tile([128, nr * K], f32, tag="pb", name="psup")
                        for ht in range(HT):
                            nc.tensor.matmul(
                                ps[:],
                                inpsT_sb[:, ht, nt * 128 : (nt + 1) * 128],
                                wst_sb[:, dirn * R + ri0 : dirn * R + ri0 + nr, ht, :],
                                start=(ht == 0),
                                stop=(ht == HT - 1),
                            )
                        nc.vector.tensor_copy(dst[:, ri0 : ri0 + nr, nt, :], ps[:])

            # fw supports first -> AllGather them across cores ASAP. The
            # ag_in stage is the scalar ring's first DMA so the collective's
            # data dependency clears as early as possible.
            emit_sup(1, supq_sb)
            ag_in = dramp.tile([128, R, NT, K], adt, name="ag_in", tag="ag_in")
            ag_out = dramp.tile([NC, 128, R, NT, K], adt, name="ag_out", tag="ag_out")
            nc.scalar.dma_start(ag_in[:], supq_sb[:])
            nc.gpsimd.collective_compute(
                "AllGather",
                mybir.AluOpType.bypass,
                replica_groups=[list(range(NC))],
                ins=[ag_in[:].opt()],
                outs=[ag_out[:].opt()],
            )
            nc.scalar.dma_start(biases_sb[:], biases[:, :, :])
            nc.scalar.dma_start(w1_sb[:], w1[:, :, :])
            nc.scalar.dma_start(ident_sb[:], ident[:, :])
            emit_sup(0, sup_sb)

            # ---------------- bw stream (column-parallel, kk-major) ----------
            stag = dramp.tile([NC, 2, 128, NB], f16, name="stag", tag="stag")
            rs_out = dramp.tile([1, 2, 128, NB], f16, name="rs_out", tag="rs_out")
            for mc in range(NMC):
                ats = []
                for ri in range(R):
                    at = adjp.tile([128, NT, MC], adt, tag="adj")
                    nc.sync.dma_start(at[:], adjbw[ri, mc])
                    ats.append(at)
                for kk in range(2):                         # k-half major: ps0
                    ps = psump.tile([128, MC], f32, tag="pb", name=f"ps{kk}")
                    for ri in range(R):                     # evacuates while the
                        for nt in range(NT):                # other half computes
                            lhsT = sup_sb[:, ri, nt, kk * 128 : (kk + 1) * 128]
                            for mh in range(MC // 512):
                                nc.tensor.matmul(
                                    ps[:, mh * 512 : (mh + 1) * 512],
                                    lhsT,
                                    ats[ri][:, nt, mh * 512 : (mh + 1) * 512],
                                    start=(ri == 0 and nt == 0),
                                    stop=(ri == R - 1 and nt == NT - 1),
                                )
                    ev = evacp.tile([128, MC], f16, tag="ev")
                    nc.vector.tensor_copy(ev[:], ps[:])
                    for d2 in range(MC // NB):
                        dest = (mc * MC) // NB + d2
                        nc.scalar.dma_start(
                            stag[dest, kk], ev[:, d2 * NB : (d2 + 1) * NB]
                        )
            nc.gpsimd.collective_compute(
                "ReduceScatter",
                mybir.AluOpType.add,
                replica_groups=[list(range(NC))],
                ins=[stag[:].opt()],
                outs=[rs_out[:].opt()],
            )

            # ---------------- fw stream (row-parallel over own m-block) ------
            psf = [
                psump.tile([128, NB], f32, tag="pb", name=f"psf{kk}")
                for kk in range(2)
            ]
            for c2 in range(NC):
                # sg rides the scalar ring: its AllGather-gated semaphore wait
                # must not stall the sync ring's adjacency prefetch behind it.
                sg = supagp.tile([128, R, NT, K], adt, tag="sg")
                nc.scalar.dma_start(sg[:], ag_out[c2])
                for ri in range(R):
                    at = adjfp.tile([128, NT, NB], adt, tag="adjf")
                    nc.sync.dma_start(at[:], adjfw[ri, c2])
                    for nt in range(NT):
                        for kk in range(2):
                            nc.tensor.matmul(
                                psf[kk][:],
                                sg[:, ri, nt, kk * 128 : (kk + 1) * 128],
                                at[:, nt, :],
                                start=(c2 == 0 and ri == 0 and nt == 0),
                                stop=(c2 == NC - 1 and ri == R - 1 and nt == NT - 1),
                            )

            # -------- bias + relu + final linear + residual ------------------
            # bw halves (ht 0,1) come from the RS result and can run during
            # the fw stream; fw halves (ht 2,3) read straight from PSUM.
            frelu_sb = const.tile([128, HT, NB], f32r)      # [p_h, ht, m_loc]
            ft = evacp.tile([128, 2, NB], f16, tag="ftmp")
            nc.scalar.dma_start(ft[:], rs_out[0].rearrange("k p n -> p k n"))
            psos = [
                psum2.tile([128, NB], f32, tag="pso", name=f"pso{jt}")
                for jt in range(JT)
            ]
            for ht in range(HT):                            # ht -> (dir, k-half)
                if ht < 2:
                    nc.scalar.activation(
                        frelu_sb[:, ht, :],
                        ft[:, ht, :],
                        Relu,
                        bias=biases_sb[:, ht, 0:1],
                    )
                else:
                    nc.scalar.activation(
                        frelu_sb[:, ht, :],
                        psf[ht - 2][:],
                        Relu,
                        scale=1.0 / ASCALE,
                        bias=biases_sb[:, ht, 0:1],
                    )
                for jt in range(JT):
                    nc.tensor.matmul(
                        psos[jt][:],
                        w1_sb[:, ht, jt * 128 : (jt + 1) * 128],
                        frelu_sb[:, ht, :],
                        start=(ht == 0),
                        stop=False,
                    )
                    if ht == HT - 1:
                        # residual: psum += I.T @ inps^T block
                        nc.tensor.matmul(
                            psos[jt][:],
                            ident_sb[:],
                            inpsT_sb[:, jt, :],
                            start=False,
                            stop=True,
                        )
            for jt in range(JT):
                ot = evacp.tile([128, NB], f16, tag="ev")
                nc.scalar.activation(
                    ot[:], psos[jt][:], Identity, bias=biases_sb[:, jt, 1:2]
                )
                nc.sync.dma_start(outT[:, jt, :], ot[:])

    nc.compile()
    nc.finalize()
    _BUILT["nc"] = nc
    return nc


def _round_fp32r(a):
    """Round fp32 to the fp32r (TF32-like, 1s+8e+11m in top 20 bits) format
    with round-to-nearest-even, as the PE's fp32r datapath expects."""
    b = np.ascontiguousarray(a, np.float32).view(np.uint32).astype(np.uint64)
    lsb = (b >> 12) & 1
    r = ((b + 0x7FF + lsb) & 0xFFFFF000).astype(np.uint32)
    return r.view(np.float32)


def _make_in_maps(inps, fw_adjs, bw_adjs, W_fw, b_fw, W_bw, b_bw, W1, b1):
    f = np.float32
    adt = ml_dtypes.float8_e3m4 if ADJ_MODE == "f8e3" else np.float16
    inps = np.asarray(inps, f)
    W1p = np.ascontiguousarray(
        _round_fp32r(np.asarray(W1, f)).reshape(HT, 128, H).transpose(1, 0, 2)
    )
    # stacked [bw/16, fw] (bw pre-scaled for the fp8 16x domain; fw supports
    # are e3m4-gathered at natural scale and rescaled in the fw relu),
    # swizzled to [p, r, ht, k]
    wcat = np.concatenate(
        [np.asarray(W_bw, f) / ASCALE, np.asarray(W_fw, f)], axis=0
    )
    wst = np.ascontiguousarray(
        wcat.reshape(2 * R, HT, 128, K).transpose(2, 0, 1, 3), np.float16
    )
    # pre-relu bias: sum_r b_r + 0.5 * colsum(inps) @ sum_r W_r  (concat bw|fw)
    colsum = inps.sum(axis=0)                                 # [H]
    pre_bias = np.concatenate(
        [
            np.asarray(b_bw, f).sum(axis=0) + 0.5 * (colsum @ np.asarray(W_bw, f).sum(axis=0)),
            np.asarray(b_fw, f).sum(axis=0) + 0.5 * (colsum @ np.asarray(W_fw, f).sum(axis=0)),
        ]
    )                                                         # [H]
    biases = np.empty((128, JT, 2), f)
    biases[:, :, 0] = pre_bias.reshape(JT, 128).T
    biases[:, :, 1] = np.asarray(b1, f).reshape(JT, 128).T
    biases = np.ascontiguousarray(biases)

    fw_adjs = np.asarray(fw_adjs, f)
    bw_adjs = np.asarray(bw_adjs, f)

    def q(blk):
        if ADJ_MODE == "f8e3":
            blk = (blk - 0.5) * ASCALE
        return blk.astype(adt)

    in_maps = []
    for c in range(NC):
        sl = slice(c * NB, (c + 1) * NB)
        # bw: column-parallel — contraction slice n in sl, all m.
        adjbw_c = np.empty((R, NMC, 128, NT, MC), adt)
        for r in range(R):
            blk = q(bw_adjs[r][:, sl].T)                      # [n_loc, m]
            adjbw_c[r] = blk.reshape(NT, 128, NMC, MC).transpose(2, 1, 0, 3)
        # fw: row-parallel — own m rows in sl, all n (transposed).
        adjfw_c = np.empty((R, NC, 128, NT, NB), adt)
        for r in range(R):
            blk = q(fw_adjs[r][sl, :].T)                      # [n_all, m_own]
            adjfw_c[r] = blk.reshape(NC, NT, 128, NB).transpose(0, 2, 1, 3)
        in_maps.append(
            {
                "inpsT": np.ascontiguousarray(
                    inps[sl].T.reshape(HT, 128, NB).transpose(1, 0, 2), np.float16
                ),
                "wst": wst,
                "adjbw": adjbw_c,
                "adjfw": adjfw_c,
                "biases": biases,
                "w1": W1p,
                "ident": np.eye(128, dtype=np.float16),
            }
        )
    return in_maps


def run(trace=False, **inputs):
    """Run the SPMD kernel; returns (full_output, BassKernelResults)."""
    from concourse.bass_utils import run_bass_kernel_spmd

    nc = _build_nc()
    in_maps = _make_in_maps(**inputs)
    res = run_bass_kernel_spmd(nc, in_maps, core_ids=list(range(NC)), trace=trace)
    out = np.empty((N, H), np.float32)
    for c in range(NC):
        # outT [p, jt, n] -> out rows [n, h=jt*128+p]
        out[c * NB : (c + 1) * NB] = (
            res.results[c]["outT"].astype(np.float32).transpose(2, 1, 0).reshape(NB, H)
        )
    return out, res


def kernel(**inputs):
    # Collective-heavy SPMD runs have shown a rare corrupted execution
    # (launch-skew related). Executions are cheap next to compile, so run
    # twice and accept only agreeing results, with a third as tiebreaker.
    out1, _ = run(trace=False, **inputs)
    out2, _ = run(trace=False, **inputs)
    if np.array_equal(out1, out2):
        return out1
    out3, _ = run(trace=False, **inputs)
    return out3 if np.array_equal(out2, out3) else out1
